# revision 19
# baseline (speedup 1.0000x reference)
"""CLS-AttentionPool2d Trainium2 kernel (8 NeuronCores, data-parallel over batch).

Math refactoring (single CLS query => tiny attention):
  xt_j = x[b,:,j] + pos_j (host-fused)         (j = 0..1023, native [C, HW] layout)
  mean = xt.mean(j);  cls = mean + (pos0 - meanpos)
  q  = alpha * (Wq @ cls + bq) / sqrt(C)       (alpha=1024 so m fits fp8 range)
  qblk[k, (s,h)] = q_s[k] * [head(k) == h]     (block-diag arrangement)
  m  = Wk.T @ qblk  (fp8)                      # per-head key-projected query
  scores[slot, j]   = m.T @ xt                 (fp8 matmuls)
  scores[slot, cls] = rowmean(token scores) + qblk . kpc   (kpc = Wk@(pos0-meanpos))
  p = softmax(scores / alpha)  ;  p' = p_tok + p_cls/1024  (folds CLS-mean)
  w  = p'.T @ xt_transposed (bf16) + p_cls * (pos0 - meanpos)
  out[c'] = sum_c Wv[c',c] * w[head(c'), c] + bv   (select by OUTPUT head)

x(+pos) is shipped from the host in BOTH layouts: natural [c, j] in fp8e4
(scores+means; softmax washes out fp8 noise) and transposed [j, c] in bf16
(weighted sum; fp8 there would break the 2e-2 error budget). This removes the
on-device xbar transpose and cuts HBM reads ~3x vs f32.

The two batch-groups are software-pipelined with explicit phase interleaving
so the PE queue stays fed while softmax/means run on scalar/vector engines.
"""

import math
import numpy as np

import concourse.bass as bass
import concourse.mybir as mybir
import concourse.tile as tile
from concourse import bacc
from concourse.bass import ts
from concourse.bass_utils import run_bass_kernel_spmd

F32 = mybir.dt.float32
BF16 = mybir.dt.bfloat16
FP8 = mybir.dt.float8e4
AX = mybir.AxisListType
ALU = mybir.AluOpType
ACTF = mybir.ActivationFunctionType

B, C, HW = 64, 512, 1024
NH, DH = 8, 64
NCORES = 8
BPC = B // NCORES          # 8 batches per core
GRP = 4                    # batches per group (2 groups per core)
NGRP = BPC // GRP
CT = C // 128              # 4 c-chunks
JT = HW // 128             # 8 j-chunks
ISQ = 1.0 / math.sqrt(C)
ALPHA = 1024.0             # q scale so m ~ O(1) in fp8

_CACHE = {}


def _build_nc():
    nc = bacc.Bacc("TRN2", target_bir_lowering=False, debug=False,
                   num_devices=NCORES)

    # ---- DRAM I/O ----
    xs = nc.dram_tensor("xs", [BPC, C, HW], BF16, kind="ExternalInput")
    xtr = nc.dram_tensor("xtr", [BPC, 128, CT, JT, 128], BF16,
                         kind="ExternalInput")
    wqt = nc.dram_tensor("wqt", [128, CT, C], BF16, kind="ExternalInput")
    wk = nc.dram_tensor("wk", [128, CT, C], BF16, kind="ExternalInput")
    wvt2 = nc.dram_tensor("wvt2", [128, CT, C], BF16, kind="ExternalInput")
    kpc = nc.dram_tensor("kpc", [128, CT], BF16, kind="ExternalInput")
    pos0row = nc.dram_tensor("pos0row", [1, C], BF16, kind="ExternalInput")
    pos0 = nc.dram_tensor("pos0", [128, CT], F32, kind="ExternalInput")
    bqs = nc.dram_tensor("bqs", [128, CT], F32, kind="ExternalInput")
    bv = nc.dram_tensor("bv", [128, CT], F32, kind="ExternalInput")
    mask32 = nc.dram_tensor("mask32", [128, CT, 32], F32, kind="ExternalInput")
    ident = nc.dram_tensor("ident", [128, 128], BF16, kind="ExternalInput")
    out_d = nc.dram_tensor("out", [BPC, C], F32, kind="ExternalOutput")

    with tile.TileContext(nc) as tc:
        with (
            tc.tile_pool(name="persist", bufs=1) as pp,
            tc.tile_pool(name="bignat", bufs=8) as bigp,
            tc.tile_pool(name="bigtr", bufs=8) as bigtp,
            tc.tile_pool(name="work", bufs=2) as wp,
            tc.tile_pool(name="junkp", bufs=4) as jp,
            tc.tile_pool(name="psA", bufs=2, space="PSUM") as psA,
            tc.tile_pool(name="psB", bufs=2, space="PSUM") as psB,
            tc.tile_pool(name="psC", bufs=1, space="PSUM") as psC,
            tc.tile_pool(name="psD", bufs=1, space="PSUM") as psD,
        ):
            # persistent tiles
            wqt_s = pp.tile([128, CT, C], BF16)
            wk_s = pp.tile([128, CT, C], BF16)
            wvt2_s = pp.tile([128, CT, C], BF16)
            kpc_s = pp.tile([128, CT], BF16)
            pos0row_s = pp.tile([1, C], BF16)
            pos0_s = pp.tile([128, CT], F32)
            bqs_s = pp.tile([128, CT], F32)
            bv_s = pp.tile([128, CT], F32)
            mask_s = pp.tile([128, CT, 32], F32)
            ident_s = pp.tile([128, 128], BF16)

            # ---- all input DMAs on one queue, in priority order ----
            xb_all = [[bigp.tile([128, CT, HW], BF16, tag="big",
                                 name=f"xb{g}_{s}")
                       for s in range(GRP)] for g in range(NGRP)]
            xtb_all = [[bigtp.tile([128, CT, JT, 128], BF16, tag="bigtr",
                                   name=f"xtb{g}_{s}")
                        for s in range(GRP)] for g in range(NGRP)]

            def load_nat(g):
                for s in range(GRP):
                    nc.sync.dma_start(
                        out=xb_all[g][s][:],
                        in_=xs[g * GRP + s].rearrange("(t p) j -> p t j",
                                                      p=128))

            def load_tr(g):
                for s in range(GRP):
                    nc.sync.dma_start(out=xtb_all[g][s][:],
                                      in_=xtr[g * GRP + s])

            load_nat(0)
            for dst, src in [(mask_s, mask32), (pos0_s, pos0),
                             (bqs_s, bqs), (wqt_s, wqt),
                             (kpc_s, kpc), (ident_s, ident)]:
                nc.sync.dma_start(out=dst[:], in_=src[:])
            load_nat(1)
            load_tr(0)
            nc.sync.dma_start(out=wk_s[:], in_=wk[:])
            for dst, src in [(pos0row_s, pos0row), (bv_s, bv),
                             (wvt2_s, wvt2)]:
                nc.sync.dma_start(out=dst[:], in_=src[:])
            load_tr(1)

            # per-group state
            st = [dict() for _ in range(NGRP)]

            def means(g):
                xb = xb_all[g]
                sums = wp.tile([128, CT, GRP], F32, tag="sums")
                junk = jp.tile([128, HW], BF16, tag="junk")
                st[g]["sums"] = sums
                # per batch: chunks 0-1 on scalar, 2-3 on vector (latency /2)
                for s in range(GRP):
                    for t in range(2):
                        nc.scalar.activation(
                            junk[:], xb[s][:, t, :], ACTF.Copy,
                            scale=1.0,
                            accum_out=sums[:, t, s:s + 1])
                    nc.vector.tensor_reduce(
                        sums[:, 2:4, s:s + 1].rearrange("p t one -> p (t one)"),
                        xb[s][:, 2:4, :], axis=AX.X, op=ALU.add)

            def qm(g):
                sums = st[g]["sums"]
                cls_all = wp.tile([128, CT, GRP], BF16, tag="cls")
                nc.vector.scalar_tensor_tensor(
                    out=cls_all[:], in0=sums[:], scalar=1.0 / HW,
                    in1=pos0_s[:, :, None].broadcast_to([128, CT, GRP]),
                    op0=ALU.mult, op1=ALU.add)

                q_ps = psC.tile([128, CT, GRP], F32, tag="psC")
                for mc in range(CT):
                    for tk in range(CT):
                        nc.tensor.matmul(
                            q_ps[:, mc, :], wqt_s[:, tk, ts(mc, 128)],
                            cls_all[:, tk, :],
                            start=(tk == 0), stop=(tk == CT - 1))
                q_sb = wp.tile([128, CT, GRP], F32, tag="qsb")
                nc.vector.scalar_tensor_tensor(
                    out=q_sb[:], in0=q_ps[:], scalar=ISQ * ALPHA,
                    in1=bqs_s[:, :, None].broadcast_to([128, CT, GRP]),
                    op0=ALU.mult, op1=ALU.add)

                qblk = wp.tile([128, CT, GRP, 32], BF16, tag="qblk")
                nc.vector.tensor_mul(
                    qblk[:],
                    q_sb[:, :, :, None].broadcast_to([128, CT, GRP, 32]),
                    mask_s[:, :, None, :].broadcast_to([128, CT, GRP, 32]))
                qblk_f = qblk[:].rearrange("p t s u -> p t (s u)")
                st[g]["qblk_f"] = qblk_f

                m_ps = psC.tile([128, CT, 128], F32, tag="psC")
                for mc in range(CT):
                    for tk in range(CT):
                        nc.tensor.matmul(
                            m_ps[:, mc, :], wk_s[:, tk, ts(mc, 128)],
                            qblk_f[:, tk, :],
                            start=(tk == 0), stop=(tk == CT - 1))
                m_sb = wp.tile([128, CT, 128], BF16, tag="msb")
                with nc.allow_low_precision(reason="fp8 scores by design"):
                    nc.vector.tensor_copy(m_sb[:], m_ps[:])
                st[g]["m_sb"] = m_sb

                # CLS score: qblk . kpc (rowmean of token scores added later)
                sccls_ps = psB.tile([128, 1], F32, tag="psB")
                for tk in range(CT):
                    nc.tensor.matmul(
                        sccls_ps[:], qblk_f[:, tk, :], kpc_s[:, tk, None],
                        start=(tk == 0), stop=(tk == CT - 1))
                # move to SBUF immediately: keeps psB rotation free for pT/wT
                sccls_sb = wp.tile([128, 1], F32, tag="scclssb")
                nc.vector.tensor_copy(sccls_sb[:], sccls_ps[:])
                st[g]["sccls"] = sccls_sb

            def scores(g):
                xb = xb_all[g]
                m_sb = st[g]["m_sb"]
                sc_ps = psA.tile([128, 2, 512], F32, tag="psA")
                st[g]["sc_ps"] = sc_ps
                # 32-row dst zero-inits the u>=8 junk slots (m cols are 0)
                for s in range(GRP):
                    for tk in range(CT):
                        for jc in range(2):
                            nc.tensor.matmul(
                                sc_ps[32 * s:32 * s + 32, jc, :],
                                m_sb[:, tk, 32 * s:32 * s + 32],
                                xb[s][:, tk, ts(jc, 512)],
                                start=(tk == 0), stop=(tk == CT - 1),
                                tile_position=(0, 32 * s),
                                skip_group_check=True)

            def softmax(g):
                sc_ps = st[g]["sc_ps"]
                sccls = st[g]["sccls"]
                redcol = wp.tile([128, 1], F32, tag="redcol")
                nc.vector.reduce_sum(redcol[:], sc_ps[:], axis=AX.XY)
                nc.vector.scalar_tensor_tensor(
                    out=sccls[:], in0=redcol[:], scalar=1.0 / HW,
                    in1=sccls[:], op0=ALU.mult, op1=ALU.add)

                p_sb = wp.tile([128, HW + 1], BF16, tag="psb")
                sumexp = wp.tile([128, 1], F32, tag="sumexp")
                se2 = wp.tile([128, 1], F32, tag="se2")
                nc.scalar.activation(p_sb[:, 0:HW], sc_ps[:], ACTF.Exp,
                                     scale=1.0 / ALPHA, accum_out=sumexp[:])
                nc.scalar.activation(p_sb[:, HW:HW + 1], sccls[:],
                                     ACTF.Exp, scale=1.0 / ALPHA,
                                     accum_out=se2[:])
                nc.vector.tensor_add(sumexp[:], sumexp[:], se2[:])
                rz = wp.tile([128, 1], F32, tag="rz")
                nc.vector.reciprocal(rz[:], sumexp[:])
                pcls_sc = wp.tile([128, 1], F32, tag="pclssc")
                nc.vector.tensor_scalar_mul(pcls_sc[:], p_sb[:, HW:HW + 1],
                                            1.0 / HW)
                nc.vector.tensor_scalar_add(p_sb[:, 0:HW], p_sb[:, 0:HW],
                                            pcls_sc[:])
                st[g]["p_sb"] = p_sb
                st[g]["rz"] = rz

            def ptrans(g):
                p_sb = st[g]["p_sb"]
                pT = wp.tile([128, JT, 128], BF16, tag="pT")
                for half in range(2):
                    tp = psB.tile([128, 512], BF16, tag="psB")
                    for k in range(4):
                        jc = half * 4 + k
                        nc.tensor.transpose(tp[:, ts(k, 128)],
                                            p_sb[:, ts(jc, 128)], ident_s[:])
                    nc.vector.tensor_copy(
                        pT[:].rearrange("p j c -> p (j c)")
                        [:, half * 512:(half + 1) * 512],
                        tp[:])
                pTc_ps = psB.tile([1, 128], F32, tag="psB")
                nc.tensor.matmul(pTc_ps[:], p_sb[:, HW:HW + 1], ident_s[:],
                                 start=True, stop=True)
                pTc = wp.tile([1, 128], BF16, tag="pTc")
                nc.vector.tensor_copy(pTc[:], pTc_ps[:])
                st[g]["pT"] = pT
                st[g]["pTc"] = pTc

            def wsum(g):
                xtb = xtb_all[g]
                pT = st[g]["pT"]
                pTc = st[g]["pTc"]
                w_ps = psD.tile([128, C], F32, tag="psD")
                for s in range(GRP):
                    for jc in range(JT):
                        nc.tensor.matmul(
                            w_ps[32 * s:32 * s + 32, :],
                            pT[:, jc, 32 * s:32 * s + 32], xtb[s][:, :, jc, :],
                            start=(jc == 0), stop=(jc == JT - 1),
                            tile_position=(0, 32 * s))
                nc.tensor.matmul(w_ps[:], pTc[:], pos0row_s[:],
                                 start=False, stop=True, skip_group_check=True)
                w_sb = wp.tile([128, C], BF16, tag="wsb")
                nc.vector.tensor_scalar_mul(w_sb[:], w_ps[:], st[g]["rz"][:])
                st[g]["w_sb"] = w_sb

            def wout(g):
                w_sb = st[g]["w_sb"]
                tp3 = psB.tile([128, 512], BF16, tag="psB")
                for mc in range(CT):
                    nc.tensor.transpose(tp3[:, ts(mc, 128)],
                                        w_sb[:, ts(mc, 128)], ident_s[:])
                wt_sb = wp.tile([128, CT, GRP, 32], BF16, tag="wtsb")
                nc.vector.tensor_copy(
                    wt_sb[:].rearrange("p t s u -> p (t s u)"), tp3[:])

                out_ps = psD.tile([128, CT, GRP, NH], F32, tag="psD")
                for mc in range(CT):
                    for tk in range(CT):
                        nc.tensor.matmul(
                            out_ps[:, mc], wvt2_s[:, tk, ts(mc, 128)],
                            wt_sb[:, tk, :, 0:NH],
                            start=(tk == 0), stop=(tk == CT - 1))
                om = wp.tile([128, CT, GRP, NH], F32, tag="om")
                nc.vector.tensor_mul(
                    om[:], out_ps[:],
                    mask_s[:, :, None, 0:NH].broadcast_to([128, CT, GRP, NH]))
                osel = wp.tile([128, CT, GRP], F32, tag="osel")
                nc.vector.tensor_reduce(osel[:], om[:], axis=AX.X, op=ALU.add)
                out_sb = wp.tile([128, CT, GRP], F32, tag="outsb")
                nc.vector.tensor_add(
                    out_sb[:], osel[:],
                    bv_s[:, :, None].broadcast_to([128, CT, GRP]))
                for s in range(GRP):
                    nc.sync.dma_start(
                        out=out_d[g * GRP + s].rearrange("(t p) -> p t", p=128),
                        in_=out_sb[:, :, s])

            # ---- software-pipelined emission of the two groups ----
            means(0)
            qm(0)
            means(1)
            scores(0)
            softmax(0)
            qm(1)
            scores(1)
            ptrans(0)
            wsum(0)
            softmax(1)
            wout(0)
            ptrans(1)
            wsum(1)
            wout(1)

    nc.compile()
    return nc


def _prep(pos_emb, Wq, bq, Wk, bk, Wv, bv):
    import ml_dtypes
    bf = ml_dtypes.bfloat16

    def ptn(v):  # [512] -> [128, CT], c = t*128 + p
        return np.ascontiguousarray(v.reshape(CT, 128).T)

    def chunkk(w):  # [512, N] -> [128, CT, N], k = t*128 + p
        return np.ascontiguousarray(w.reshape(CT, 128, -1).transpose(1, 0, 2))

    p1 = pos_emb[1:].sum(axis=0)
    pos0adj = (pos_emb[0] - p1 / HW).astype(np.float32)
    kpcv = (Wk.astype(np.float64) @ pos0adj.astype(np.float64)).astype(np.float32)
    mask = np.zeros((128, CT, 32), np.float32)
    for p in range(128):
        for t in range(CT):
            h = (t * 128 + p) // DH
            mask[p, t, h] = 1.0

    return {
        "wqt": chunkk(np.ascontiguousarray(Wq.T)).astype(bf),
        "wk": chunkk(Wk).astype(bf),
        "wvt2": chunkk(np.ascontiguousarray(Wv.T)).astype(bf),
        "kpc": ptn(kpcv).astype(bf),
        "pos0row": np.ascontiguousarray(pos0adj.reshape(1, C)).astype(bf),
        "pos0": ptn(pos0adj),
        "bqs": ptn(bq * ISQ * ALPHA),
        "bv": ptn(bv),
        "mask32": mask,
        "ident": np.eye(128, dtype=np.float32).astype(bf),
    }


def _make_in_maps(x, pos_emb, Wq, bq, Wk, bk, Wv, bv):
    import ml_dtypes
    bf = ml_dtypes.bfloat16
    x = np.asarray(x, dtype=np.float32).reshape(B, C, HW)
    pos_emb = np.asarray(pos_emb, np.float32)
    shared = _prep(pos_emb, np.asarray(Wq, np.float32),
                   np.asarray(bq, np.float32), np.asarray(Wk, np.float32),
                   np.asarray(bk, np.float32), np.asarray(Wv, np.float32),
                   np.asarray(bv, np.float32))
    # fold pos_emb into the tokens: xt[c, j] = x[c, j] + pos_emb[1 + j, c]
    xp = x + pos_emb[1:].T[None, :, :]
    x16 = xp.astype(bf)
    # xtr[b][p, t, jc, c'] = xp[b, t*128 + c', jc*128 + p]
    xtr = np.ascontiguousarray(
        x16.reshape(B, CT, 128, JT, 128).transpose(0, 4, 1, 3, 2))
    in_maps = []
    for i in range(NCORES):
        m = dict(shared)
        m["xs"] = np.ascontiguousarray(x16[i * BPC:(i + 1) * BPC])
        m["xtr"] = np.ascontiguousarray(xtr[i * BPC:(i + 1) * BPC])
        in_maps.append(m)
    return in_maps


def kernel(x, pos_emb, Wq, bq, Wk, bk, Wv, bv, num_heads):
    assert int(num_heads) == NH
    if "nc" not in _CACHE:
        _CACHE["nc"] = _build_nc()
    nc = _CACHE["nc"]
    in_maps = _make_in_maps(x, pos_emb, Wq, bq, Wk, bk, Wv, bv)
    res = run_bass_kernel_spmd(nc, in_maps, list(range(NCORES)))
    out = np.concatenate([res.results[i]["out"] for i in range(NCORES)], axis=0)
    return out.astype(np.float32)


# revision 21
# speedup vs baseline: 1.0034x; 1.0034x over previous
"""CLS-AttentionPool2d Trainium2 kernel (8 NeuronCores, data-parallel over batch).

Math refactoring (single CLS query => tiny attention):
  xt_j = x[b,:,j] + pos_j (host-fused)         (j = 0..1023, native [C, HW] layout)
  mean = xt.mean(j);  cls = mean + (pos0 - meanpos)
  q  = alpha * (Wq @ cls + bq) / sqrt(C)       (alpha=1024 so m fits fp8 range)
  qblk[k, (s,h)] = q_s[k] * [head(k) == h]     (block-diag arrangement)
  m  = Wk.T @ qblk  (fp8)                      # per-head key-projected query
  scores[slot, j]   = m.T @ xt                 (fp8 matmuls)
  scores[slot, cls] = rowmean(token scores) + qblk . kpc   (kpc = Wk@(pos0-meanpos))
  p = softmax(scores / alpha)  ;  p' = p_tok + p_cls/1024  (folds CLS-mean)
  w  = p'.T @ xt_transposed (bf16) + p_cls * (pos0 - meanpos)
  out[c'] = sum_c Wv[c',c] * w[head(c'), c] + bv   (select by OUTPUT head)

x(+pos) is shipped from the host in BOTH layouts: natural [c, j] in fp8e4
(scores+means; softmax washes out fp8 noise) and transposed [j, c] in bf16
(weighted sum; fp8 there would break the 2e-2 error budget). This removes the
on-device xbar transpose and cuts HBM reads ~3x vs f32.

The two batch-groups are software-pipelined with explicit phase interleaving
so the PE queue stays fed while softmax/means run on scalar/vector engines.
"""

import math
import numpy as np

import concourse.bass as bass
import concourse.mybir as mybir
import concourse.tile as tile
from concourse import bacc
from concourse.bass import ts
from concourse.bass_utils import run_bass_kernel_spmd

F32 = mybir.dt.float32
BF16 = mybir.dt.bfloat16
FP8 = mybir.dt.float8e4
AX = mybir.AxisListType
ALU = mybir.AluOpType
ACTF = mybir.ActivationFunctionType

B, C, HW = 64, 512, 1024
NH, DH = 8, 64
NCORES = 8
BPC = B // NCORES          # 8 batches per core
GRP = 4                    # batches per group (2 groups per core)
NGRP = BPC // GRP
CT = C // 128              # 4 c-chunks
JT = HW // 128             # 8 j-chunks
ISQ = 1.0 / math.sqrt(C)
ALPHA = 1024.0             # q scale so m ~ O(1) in fp8

_CACHE = {}


def _build_nc():
    nc = bacc.Bacc("TRN2", target_bir_lowering=False, debug=False,
                   num_devices=NCORES)

    # ---- DRAM I/O ----
    xs = nc.dram_tensor("xs", [BPC, C, HW], BF16, kind="ExternalInput")
    xtr = nc.dram_tensor("xtr", [BPC, 128, CT, JT, 128], BF16,
                         kind="ExternalInput")
    wqt = nc.dram_tensor("wqt", [128, CT, C], BF16, kind="ExternalInput")
    wk = nc.dram_tensor("wk", [128, CT, C], BF16, kind="ExternalInput")
    wvt2 = nc.dram_tensor("wvt2", [128, CT, C], BF16, kind="ExternalInput")
    kpc = nc.dram_tensor("kpc", [128, CT], BF16, kind="ExternalInput")
    pos0row = nc.dram_tensor("pos0row", [1, C], BF16, kind="ExternalInput")
    pos0 = nc.dram_tensor("pos0", [128, CT], F32, kind="ExternalInput")
    bqs = nc.dram_tensor("bqs", [128, CT], F32, kind="ExternalInput")
    bv = nc.dram_tensor("bv", [128, CT], F32, kind="ExternalInput")
    mask32 = nc.dram_tensor("mask32", [128, CT, 32], F32, kind="ExternalInput")
    ident = nc.dram_tensor("ident", [128, 128], BF16, kind="ExternalInput")
    out_d = nc.dram_tensor("out", [BPC, C], F32, kind="ExternalOutput")

    with tile.TileContext(nc) as tc:
        with (
            tc.tile_pool(name="persist", bufs=1) as pp,
            tc.tile_pool(name="bignat", bufs=8) as bigp,
            tc.tile_pool(name="bigtr", bufs=8) as bigtp,
            tc.tile_pool(name="work", bufs=2) as wp,
            tc.tile_pool(name="junkp", bufs=4) as jp,
            tc.tile_pool(name="psA", bufs=2, space="PSUM") as psA,
            tc.tile_pool(name="psB", bufs=2, space="PSUM") as psB,
            tc.tile_pool(name="psC", bufs=1, space="PSUM") as psC,
            tc.tile_pool(name="psD", bufs=1, space="PSUM") as psD,
        ):
            # persistent tiles
            wqt_s = pp.tile([128, CT, C], BF16)
            wk_s = pp.tile([128, CT, C], BF16)
            wvt2_s = pp.tile([128, CT, C], BF16)
            kpc_s = pp.tile([128, CT], BF16)
            pos0row_s = pp.tile([1, C], BF16)
            pos0_s = pp.tile([128, CT], F32)
            bqs_s = pp.tile([128, CT], F32)
            bv_s = pp.tile([128, CT], F32)
            mask_s = pp.tile([128, CT, 32], F32)
            ident_s = pp.tile([128, 128], BF16)

            # ---- all input DMAs on one queue, in priority order ----
            xb_all = [[bigp.tile([128, CT, HW], BF16, tag="big",
                                 name=f"xb{g}_{s}")
                       for s in range(GRP)] for g in range(NGRP)]
            xtb_all = [[bigtp.tile([128, CT, JT, 128], BF16, tag="bigtr",
                                   name=f"xtb{g}_{s}")
                        for s in range(GRP)] for g in range(NGRP)]

            def load_nat(g):
                for s in range(GRP):
                    nc.sync.dma_start(
                        out=xb_all[g][s][:],
                        in_=xs[g * GRP + s].rearrange("(t p) j -> p t j",
                                                      p=128))

            def load_tr(g):
                for s in range(GRP):
                    nc.sync.dma_start(out=xtb_all[g][s][:],
                                      in_=xtr[g * GRP + s])

            load_nat(0)
            for dst, src in [(mask_s, mask32), (pos0_s, pos0),
                             (bqs_s, bqs), (wqt_s, wqt),
                             (kpc_s, kpc), (ident_s, ident),
                             (wk_s, wk)]:
                nc.sync.dma_start(out=dst[:], in_=src[:])
            # interleave nat-g1 (means g1) with tr-g0 (wsum g0) per batch
            for s in range(GRP):
                nc.sync.dma_start(
                    out=xb_all[1][s][:],
                    in_=xs[GRP + s].rearrange("(t p) j -> p t j", p=128))
                nc.sync.dma_start(out=xtb_all[0][s][:], in_=xtr[s])
            for dst, src in [(pos0row_s, pos0row), (bv_s, bv),
                             (wvt2_s, wvt2)]:
                nc.sync.dma_start(out=dst[:], in_=src[:])
            load_tr(1)

            # per-group state
            st = [dict() for _ in range(NGRP)]

            def means(g):
                xb = xb_all[g]
                sums = wp.tile([128, CT, GRP], F32, tag="sums")
                junk = jp.tile([128, HW], BF16, tag="junk")
                st[g]["sums"] = sums
                # per batch: chunks 0-1 on scalar, 2-3 on vector (latency /2)
                for s in range(GRP):
                    for t in range(2):
                        nc.scalar.activation(
                            junk[:], xb[s][:, t, :], ACTF.Copy,
                            scale=1.0,
                            accum_out=sums[:, t, s:s + 1])
                    nc.vector.tensor_reduce(
                        sums[:, 2:4, s:s + 1].rearrange("p t one -> p (t one)"),
                        xb[s][:, 2:4, :], axis=AX.X, op=ALU.add)

            def qm(g):
                sums = st[g]["sums"]
                cls_all = wp.tile([128, CT, GRP], BF16, tag="cls")
                nc.vector.scalar_tensor_tensor(
                    out=cls_all[:], in0=sums[:], scalar=1.0 / HW,
                    in1=pos0_s[:, :, None].broadcast_to([128, CT, GRP]),
                    op0=ALU.mult, op1=ALU.add)

                q_ps = psC.tile([128, CT, GRP], F32, tag="psC")
                for mc in range(CT):
                    for tk in range(CT):
                        nc.tensor.matmul(
                            q_ps[:, mc, :], wqt_s[:, tk, ts(mc, 128)],
                            cls_all[:, tk, :],
                            start=(tk == 0), stop=(tk == CT - 1))
                q_sb = wp.tile([128, CT, GRP], F32, tag="qsb")
                nc.vector.scalar_tensor_tensor(
                    out=q_sb[:], in0=q_ps[:], scalar=ISQ * ALPHA,
                    in1=bqs_s[:, :, None].broadcast_to([128, CT, GRP]),
                    op0=ALU.mult, op1=ALU.add)

                qblk = wp.tile([128, CT, GRP, 32], BF16, tag="qblk")
                nc.vector.tensor_mul(
                    qblk[:],
                    q_sb[:, :, :, None].broadcast_to([128, CT, GRP, 32]),
                    mask_s[:, :, None, :].broadcast_to([128, CT, GRP, 32]))
                qblk_f = qblk[:].rearrange("p t s u -> p t (s u)")
                st[g]["qblk_f"] = qblk_f

                m_ps = psC.tile([128, CT, 128], F32, tag="psC")
                for mc in range(CT):
                    for tk in range(CT):
                        nc.tensor.matmul(
                            m_ps[:, mc, :], wk_s[:, tk, ts(mc, 128)],
                            qblk_f[:, tk, :],
                            start=(tk == 0), stop=(tk == CT - 1))
                m_sb = wp.tile([128, CT, 128], BF16, tag="msb")
                with nc.allow_low_precision(reason="fp8 scores by design"):
                    nc.vector.tensor_copy(m_sb[:], m_ps[:])
                st[g]["m_sb"] = m_sb

                # CLS score: qblk . kpc (rowmean of token scores added later)
                sccls_ps = psB.tile([128, 1], F32, tag="psB")
                for tk in range(CT):
                    nc.tensor.matmul(
                        sccls_ps[:], qblk_f[:, tk, :], kpc_s[:, tk, None],
                        start=(tk == 0), stop=(tk == CT - 1))
                # move to SBUF immediately: keeps psB rotation free for pT/wT
                sccls_sb = wp.tile([128, 1], F32, tag="scclssb")
                nc.vector.tensor_copy(sccls_sb[:], sccls_ps[:])
                st[g]["sccls"] = sccls_sb

            def scores(g):
                xb = xb_all[g]
                m_sb = st[g]["m_sb"]
                sc_ps = psA.tile([128, 2, 512], F32, tag="psA")
                st[g]["sc_ps"] = sc_ps
                # 32-row dst zero-inits the u>=8 junk slots (m cols are 0)
                for s in range(GRP):
                    for tk in range(CT):
                        for jc in range(2):
                            nc.tensor.matmul(
                                sc_ps[32 * s:32 * s + 32, jc, :],
                                m_sb[:, tk, 32 * s:32 * s + 32],
                                xb[s][:, tk, ts(jc, 512)],
                                start=(tk == 0), stop=(tk == CT - 1),
                                tile_position=(0, 32 * s),
                                skip_group_check=True)

            def softmax(g):
                sc_ps = st[g]["sc_ps"]
                sccls = st[g]["sccls"]
                p_sb = wp.tile([128, HW + 1], BF16, tag="psb")
                sumexp = wp.tile([128, 1], F32, tag="sumexp")
                se2 = wp.tile([128, 1], F32, tag="se2")
                # token exp first: redcol/stt on DVE hide under it
                nc.scalar.activation(p_sb[:, 0:HW], sc_ps[:], ACTF.Exp,
                                     scale=1.0 / ALPHA, accum_out=sumexp[:])
                redcol = wp.tile([128, 1], F32, tag="redcol")
                nc.vector.reduce_sum(redcol[:], sc_ps[:], axis=AX.XY)
                nc.vector.scalar_tensor_tensor(
                    out=sccls[:], in0=redcol[:], scalar=1.0 / HW,
                    in1=sccls[:], op0=ALU.mult, op1=ALU.add)
                nc.scalar.activation(p_sb[:, HW:HW + 1], sccls[:],
                                     ACTF.Exp, scale=1.0 / ALPHA,
                                     accum_out=se2[:])
                nc.vector.tensor_add(sumexp[:], sumexp[:], se2[:])
                rz = wp.tile([128, 1], F32, tag="rz")
                nc.vector.reciprocal(rz[:], sumexp[:])
                pcls_sc = wp.tile([128, 1], F32, tag="pclssc")
                nc.vector.tensor_scalar_mul(pcls_sc[:], p_sb[:, HW:HW + 1],
                                            1.0 / HW)
                nc.vector.tensor_scalar_add(p_sb[:, 0:HW], p_sb[:, 0:HW],
                                            pcls_sc[:])
                st[g]["p_sb"] = p_sb
                st[g]["rz"] = rz

            def ptrans(g):
                p_sb = st[g]["p_sb"]
                pT = wp.tile([128, JT, 128], BF16, tag="pT")
                for half in range(2):
                    tp = psB.tile([128, 512], BF16, tag="psB")
                    for k in range(4):
                        jc = half * 4 + k
                        nc.tensor.transpose(tp[:, ts(k, 128)],
                                            p_sb[:, ts(jc, 128)], ident_s[:])
                    nc.vector.tensor_copy(
                        pT[:].rearrange("p j c -> p (j c)")
                        [:, half * 512:(half + 1) * 512],
                        tp[:])
                pTc_ps = psB.tile([1, 128], F32, tag="psB")
                nc.tensor.matmul(pTc_ps[:], p_sb[:, HW:HW + 1], ident_s[:],
                                 start=True, stop=True)
                pTc = wp.tile([1, 128], BF16, tag="pTc")
                nc.vector.tensor_copy(pTc[:], pTc_ps[:])
                st[g]["pT"] = pT
                st[g]["pTc"] = pTc

            def wsum(g):
                xtb = xtb_all[g]
                pT = st[g]["pT"]
                pTc = st[g]["pTc"]
                w_ps = psD.tile([128, C], F32, tag="psD")
                for s in range(GRP):
                    for jc in range(JT):
                        nc.tensor.matmul(
                            w_ps[32 * s:32 * s + 32, :],
                            pT[:, jc, 32 * s:32 * s + 32], xtb[s][:, :, jc, :],
                            start=(jc == 0), stop=(jc == JT - 1),
                            tile_position=(0, 32 * s))
                nc.tensor.matmul(w_ps[:], pTc[:], pos0row_s[:],
                                 start=False, stop=True, skip_group_check=True)
                w_sb = wp.tile([128, C], BF16, tag="wsb")
                nc.vector.tensor_scalar_mul(w_sb[:], w_ps[:], st[g]["rz"][:])
                st[g]["w_sb"] = w_sb

            def wout(g):
                w_sb = st[g]["w_sb"]
                tp3 = psB.tile([128, 512], BF16, tag="psB")
                for mc in range(CT):
                    nc.tensor.transpose(tp3[:, ts(mc, 128)],
                                        w_sb[:, ts(mc, 128)], ident_s[:])
                wt_sb = wp.tile([128, CT, GRP, 32], BF16, tag="wtsb")
                nc.vector.tensor_copy(
                    wt_sb[:].rearrange("p t s u -> p (t s u)"), tp3[:])

                out_ps = psD.tile([128, CT, GRP, NH], F32, tag="psD")
                for mc in range(CT):
                    for tk in range(CT):
                        nc.tensor.matmul(
                            out_ps[:, mc], wvt2_s[:, tk, ts(mc, 128)],
                            wt_sb[:, tk, :, 0:NH],
                            start=(tk == 0), stop=(tk == CT - 1))
                om = wp.tile([128, CT, GRP, NH], F32, tag="om")
                nc.vector.tensor_mul(
                    om[:], out_ps[:],
                    mask_s[:, :, None, 0:NH].broadcast_to([128, CT, GRP, NH]))
                osel = wp.tile([128, CT, GRP], F32, tag="osel")
                nc.vector.tensor_reduce(osel[:], om[:], axis=AX.X, op=ALU.add)
                out_sb = wp.tile([128, CT, GRP], F32, tag="outsb")
                nc.vector.tensor_add(
                    out_sb[:], osel[:],
                    bv_s[:, :, None].broadcast_to([128, CT, GRP]))
                for s in range(GRP):
                    nc.sync.dma_start(
                        out=out_d[g * GRP + s].rearrange("(t p) -> p t", p=128),
                        in_=out_sb[:, :, s])

            # ---- software-pipelined emission of the two groups ----
            means(0)
            qm(0)
            means(1)
            scores(0)
            softmax(0)
            qm(1)
            scores(1)
            ptrans(0)
            wsum(0)
            softmax(1)
            wout(0)
            ptrans(1)
            wsum(1)
            wout(1)

    nc.compile()
    return nc


def _prep(pos_emb, Wq, bq, Wk, bk, Wv, bv):
    import ml_dtypes
    bf = ml_dtypes.bfloat16

    def ptn(v):  # [512] -> [128, CT], c = t*128 + p
        return np.ascontiguousarray(v.reshape(CT, 128).T)

    def chunkk(w):  # [512, N] -> [128, CT, N], k = t*128 + p
        return np.ascontiguousarray(w.reshape(CT, 128, -1).transpose(1, 0, 2))

    p1 = pos_emb[1:].sum(axis=0)
    pos0adj = (pos_emb[0] - p1 / HW).astype(np.float32)
    kpcv = (Wk.astype(np.float64) @ pos0adj.astype(np.float64)).astype(np.float32)
    mask = np.zeros((128, CT, 32), np.float32)
    for p in range(128):
        for t in range(CT):
            h = (t * 128 + p) // DH
            mask[p, t, h] = 1.0

    return {
        "wqt": chunkk(np.ascontiguousarray(Wq.T)).astype(bf),
        "wk": chunkk(Wk).astype(bf),
        "wvt2": chunkk(np.ascontiguousarray(Wv.T)).astype(bf),
        "kpc": ptn(kpcv).astype(bf),
        "pos0row": np.ascontiguousarray(pos0adj.reshape(1, C)).astype(bf),
        "pos0": ptn(pos0adj),
        "bqs": ptn(bq * ISQ * ALPHA),
        "bv": ptn(bv),
        "mask32": mask,
        "ident": np.eye(128, dtype=np.float32).astype(bf),
    }


def _make_in_maps(x, pos_emb, Wq, bq, Wk, bk, Wv, bv):
    import ml_dtypes
    bf = ml_dtypes.bfloat16
    x = np.asarray(x, dtype=np.float32).reshape(B, C, HW)
    pos_emb = np.asarray(pos_emb, np.float32)
    shared = _prep(pos_emb, np.asarray(Wq, np.float32),
                   np.asarray(bq, np.float32), np.asarray(Wk, np.float32),
                   np.asarray(bk, np.float32), np.asarray(Wv, np.float32),
                   np.asarray(bv, np.float32))
    # fold pos_emb into the tokens: xt[c, j] = x[c, j] + pos_emb[1 + j, c]
    xp = x + pos_emb[1:].T[None, :, :]
    x16 = xp.astype(bf)
    # xtr[b][p, t, jc, c'] = xp[b, t*128 + c', jc*128 + p]
    xtr = np.ascontiguousarray(
        x16.reshape(B, CT, 128, JT, 128).transpose(0, 4, 1, 3, 2))
    in_maps = []
    for i in range(NCORES):
        m = dict(shared)
        m["xs"] = np.ascontiguousarray(x16[i * BPC:(i + 1) * BPC])
        m["xtr"] = np.ascontiguousarray(xtr[i * BPC:(i + 1) * BPC])
        in_maps.append(m)
    return in_maps


def kernel(x, pos_emb, Wq, bq, Wk, bk, Wv, bv, num_heads):
    assert int(num_heads) == NH
    if "nc" not in _CACHE:
        _CACHE["nc"] = _build_nc()
    nc = _CACHE["nc"]
    in_maps = _make_in_maps(x, pos_emb, Wq, bq, Wk, bk, Wv, bv)
    res = run_bass_kernel_spmd(nc, in_maps, list(range(NCORES)))
    out = np.concatenate([res.results[i]["out"] for i in range(NCORES)], axis=0)
    return out.astype(np.float32)


# revision 22
# speedup vs baseline: 1.2659x; 1.2615x over previous
"""CLS-AttentionPool2d Trainium2 kernel (8 NeuronCores, data-parallel over batch).

Math refactoring (single CLS query => tiny attention):
  xt_j = x[b,:,j] + pos_j (host-fused)         (j = 0..1023, native [C, HW] layout)
  mean = xt.mean(j);  cls = mean + (pos0 - meanpos)
  q  = alpha * (Wq @ cls + bq) / sqrt(C)       (alpha=1024 so m fits fp8 range)
  qblk[k, (s,h)] = q_s[k] * [head(k) == h]     (block-diag arrangement)
  m  = Wk.T @ qblk  (fp8)                      # per-head key-projected query
  scores[slot, j]   = m.T @ xt                 (fp8 matmuls)
  scores[slot, cls] = rowmean(token scores) + qblk . kpc   (kpc = Wk@(pos0-meanpos))
  p = softmax(scores / alpha)  ;  p' = p_tok + p_cls/1024  (folds CLS-mean)
  w  = p'.T @ xt_transposed (bf16) + p_cls * (pos0 - meanpos)
  out[c'] = sum_c Wv[c',c] * w[head(c'), c] + bv   (select by OUTPUT head)

x(+pos) is shipped from the host in BOTH layouts: natural [c, j] in fp8e4
(scores+means; softmax washes out fp8 noise) and transposed [j, c] in bf16
(weighted sum; fp8 there would break the 2e-2 error budget). This removes the
on-device xbar transpose and cuts HBM reads ~3x vs f32.

The two batch-groups are software-pipelined with explicit phase interleaving
so the PE queue stays fed while softmax/means run on scalar/vector engines.
"""

import math
import numpy as np

import concourse.bass as bass
import concourse.mybir as mybir
import concourse.tile as tile
from concourse import bacc
from concourse.bass import ts
from concourse.bass_utils import run_bass_kernel_spmd

F32 = mybir.dt.float32
BF16 = mybir.dt.bfloat16
FP8 = mybir.dt.float8e4
AX = mybir.AxisListType
ALU = mybir.AluOpType
ACTF = mybir.ActivationFunctionType

B, C, HW = 64, 512, 1024
NH, DH = 8, 64
NCORES = 8
BPC = B // NCORES          # 8 batches per core
GRP = 4                    # batches per group (2 groups per core)
NGRP = BPC // GRP
CT = C // 128              # 4 c-chunks
JT = HW // 128             # 8 j-chunks
ISQ = 1.0 / math.sqrt(C)
ALPHA = 1024.0             # q scale so m ~ O(1) in fp8

_CACHE = {}


def _build_nc():
    nc = bacc.Bacc("TRN2", target_bir_lowering=False, debug=False,
                   num_devices=NCORES)

    # ---- DRAM I/O ----
    xs = nc.dram_tensor("xs", [BPC, C, HW], FP8, kind="ExternalInput")
    xtr = nc.dram_tensor("xtr", [BPC, 128, CT, JT, 128], BF16,
                         kind="ExternalInput")
    wqt = nc.dram_tensor("wqt", [128, CT, C], BF16, kind="ExternalInput")
    wk = nc.dram_tensor("wk", [128, CT, C], BF16, kind="ExternalInput")
    wvt2 = nc.dram_tensor("wvt2", [128, CT, C], BF16, kind="ExternalInput")
    kpc = nc.dram_tensor("kpc", [128, CT], BF16, kind="ExternalInput")
    pos0row = nc.dram_tensor("pos0row", [1, C], BF16, kind="ExternalInput")
    pos0 = nc.dram_tensor("pos0", [128, CT], F32, kind="ExternalInput")
    bqs = nc.dram_tensor("bqs", [128, CT], F32, kind="ExternalInput")
    bv = nc.dram_tensor("bv", [128, CT], F32, kind="ExternalInput")
    mask32 = nc.dram_tensor("mask32", [128, CT, 32], F32, kind="ExternalInput")
    ident = nc.dram_tensor("ident", [128, 128], BF16, kind="ExternalInput")
    out_d = nc.dram_tensor("out", [BPC, C], F32, kind="ExternalOutput")

    with tile.TileContext(nc) as tc:
        with (
            tc.tile_pool(name="persist", bufs=1) as pp,
            tc.tile_pool(name="bignat", bufs=8) as bigp,
            tc.tile_pool(name="bigtr", bufs=8) as bigtp,
            tc.tile_pool(name="work", bufs=2) as wp,
            tc.tile_pool(name="junkp", bufs=4) as jp,
            tc.tile_pool(name="psA", bufs=2, space="PSUM") as psA,
            tc.tile_pool(name="psB", bufs=2, space="PSUM") as psB,
            tc.tile_pool(name="psC", bufs=1, space="PSUM") as psC,
            tc.tile_pool(name="psD", bufs=1, space="PSUM") as psD,
        ):
            # persistent tiles
            wqt_s = pp.tile([128, CT, C], BF16)
            wk_s = pp.tile([128, CT, C], BF16)
            wvt2_s = pp.tile([128, CT, C], BF16)
            kpc_s = pp.tile([128, CT], BF16)
            pos0row_s = pp.tile([1, C], BF16)
            pos0_s = pp.tile([128, CT], F32)
            bqs_s = pp.tile([128, CT], F32)
            bv_s = pp.tile([128, CT], F32)
            mask_s = pp.tile([128, CT, 32], F32)
            ident_s = pp.tile([128, 128], BF16)

            # ---- all input DMAs on one queue, in priority order ----
            xb_all = [[bigp.tile([128, CT, HW], FP8, tag="big",
                                 name=f"xb{g}_{s}")
                       for s in range(GRP)] for g in range(NGRP)]
            xtb_all = [[bigtp.tile([128, CT, JT, 128], BF16, tag="bigtr",
                                   name=f"xtb{g}_{s}")
                        for s in range(GRP)] for g in range(NGRP)]

            def load_nat(g):
                for s in range(GRP):
                    nc.sync.dma_start(
                        out=xb_all[g][s][:],
                        in_=xs[g * GRP + s].rearrange("(t p) j -> p t j",
                                                      p=128))

            def load_tr(g):
                for s in range(GRP):
                    nc.sync.dma_start(out=xtb_all[g][s][:],
                                      in_=xtr[g * GRP + s])

            load_nat(0)
            for dst, src in [(mask_s, mask32), (pos0_s, pos0),
                             (bqs_s, bqs), (wqt_s, wqt),
                             (kpc_s, kpc), (ident_s, ident)]:
                nc.sync.dma_start(out=dst[:], in_=src[:])
            load_nat(1)
            nc.sync.dma_start(out=wk_s[:], in_=wk[:])
            load_tr(0)
            for dst, src in [(pos0row_s, pos0row), (bv_s, bv),
                             (wvt2_s, wvt2)]:
                nc.sync.dma_start(out=dst[:], in_=src[:])
            load_tr(1)

            # per-group state
            st = [dict() for _ in range(NGRP)]

            def means(g):
                xb = xb_all[g]
                sums = wp.tile([128, CT, GRP], F32, tag="sums")
                junk = jp.tile([128, HW], BF16, tag="junk")
                st[g]["sums"] = sums
                # per batch: chunks 0-1 on scalar, 2-3 on vector (latency /2)
                for s in range(GRP):
                    for t in range(2):
                        nc.scalar.activation(
                            junk[:], xb[s][:, t, :], ACTF.Copy,
                            scale=1.0,
                            accum_out=sums[:, t, s:s + 1])
                    nc.vector.tensor_reduce(
                        sums[:, 2:4, s:s + 1].rearrange("p t one -> p (t one)"),
                        xb[s][:, 2:4, :], axis=AX.X, op=ALU.add)

            def qm(g):
                sums = st[g]["sums"]
                cls_all = wp.tile([128, CT, GRP], BF16, tag="cls")
                nc.vector.scalar_tensor_tensor(
                    out=cls_all[:], in0=sums[:], scalar=1.0 / HW,
                    in1=pos0_s[:, :, None].broadcast_to([128, CT, GRP]),
                    op0=ALU.mult, op1=ALU.add)

                q_ps = psC.tile([128, CT, GRP], F32, tag="psC")
                for mc in range(CT):
                    for tk in range(CT):
                        nc.tensor.matmul(
                            q_ps[:, mc, :], wqt_s[:, tk, ts(mc, 128)],
                            cls_all[:, tk, :],
                            start=(tk == 0), stop=(tk == CT - 1))
                q_sb = wp.tile([128, CT, GRP], F32, tag="qsb")
                nc.vector.scalar_tensor_tensor(
                    out=q_sb[:], in0=q_ps[:], scalar=ISQ * ALPHA,
                    in1=bqs_s[:, :, None].broadcast_to([128, CT, GRP]),
                    op0=ALU.mult, op1=ALU.add)

                qblk = wp.tile([128, CT, GRP, 32], BF16, tag="qblk")
                nc.vector.tensor_mul(
                    qblk[:],
                    q_sb[:, :, :, None].broadcast_to([128, CT, GRP, 32]),
                    mask_s[:, :, None, :].broadcast_to([128, CT, GRP, 32]))
                qblk_f = qblk[:].rearrange("p t s u -> p t (s u)")
                st[g]["qblk_f"] = qblk_f

                m_ps = psC.tile([128, CT, 128], F32, tag="psC")
                for mc in range(CT):
                    for tk in range(CT):
                        nc.tensor.matmul(
                            m_ps[:, mc, :], wk_s[:, tk, ts(mc, 128)],
                            qblk_f[:, tk, :],
                            start=(tk == 0), stop=(tk == CT - 1))
                m_sb = wp.tile([128, CT, 128], FP8, tag="msb")
                with nc.allow_low_precision(reason="fp8 scores by design"):
                    nc.vector.tensor_copy(m_sb[:], m_ps[:])
                st[g]["m_sb"] = m_sb

                # CLS score: qblk . kpc (rowmean of token scores added later)
                sccls_ps = psB.tile([128, 1], F32, tag="psB")
                for tk in range(CT):
                    nc.tensor.matmul(
                        sccls_ps[:], qblk_f[:, tk, :], kpc_s[:, tk, None],
                        start=(tk == 0), stop=(tk == CT - 1))
                # move to SBUF immediately: keeps psB rotation free for pT/wT
                sccls_sb = wp.tile([128, 1], F32, tag="scclssb")
                nc.vector.tensor_copy(sccls_sb[:], sccls_ps[:])
                st[g]["sccls"] = sccls_sb

            def scores(g):
                xb = xb_all[g]
                m_sb = st[g]["m_sb"]
                sc_ps = psA.tile([128, 2, 512], F32, tag="psA")
                st[g]["sc_ps"] = sc_ps
                # 32-row dst zero-inits the u>=8 junk slots (m cols are 0)
                for s in range(GRP):
                    for tk in range(CT):
                        for jc in range(2):
                            nc.tensor.matmul(
                                sc_ps[32 * s:32 * s + 32, jc, :],
                                m_sb[:, tk, 32 * s:32 * s + 32],
                                xb[s][:, tk, ts(jc, 512)],
                                start=(tk == 0), stop=(tk == CT - 1),
                                tile_position=(0, 32 * s),
                                skip_group_check=True)

            def softmax(g):
                sc_ps = st[g]["sc_ps"]
                sccls = st[g]["sccls"]
                p_sb = wp.tile([128, HW + 1], BF16, tag="psb")
                sumexp = wp.tile([128, 1], F32, tag="sumexp")
                se2 = wp.tile([128, 1], F32, tag="se2")
                # token exp first: redcol/stt on DVE hide under it
                nc.scalar.activation(p_sb[:, 0:HW], sc_ps[:], ACTF.Exp,
                                     scale=1.0 / ALPHA, accum_out=sumexp[:])
                redcol = wp.tile([128, 1], F32, tag="redcol")
                nc.vector.reduce_sum(redcol[:], sc_ps[:], axis=AX.XY)
                nc.vector.scalar_tensor_tensor(
                    out=sccls[:], in0=redcol[:], scalar=1.0 / HW,
                    in1=sccls[:], op0=ALU.mult, op1=ALU.add)
                nc.scalar.activation(p_sb[:, HW:HW + 1], sccls[:],
                                     ACTF.Exp, scale=1.0 / ALPHA,
                                     accum_out=se2[:])
                nc.vector.tensor_add(sumexp[:], sumexp[:], se2[:])
                rz = wp.tile([128, 1], F32, tag="rz")
                nc.vector.reciprocal(rz[:], sumexp[:])
                pcls_sc = wp.tile([128, 1], F32, tag="pclssc")
                nc.vector.tensor_scalar_mul(pcls_sc[:], p_sb[:, HW:HW + 1],
                                            1.0 / HW)
                nc.vector.tensor_scalar_add(p_sb[:, 0:HW], p_sb[:, 0:HW],
                                            pcls_sc[:])
                st[g]["p_sb"] = p_sb
                st[g]["rz"] = rz

            def ptrans(g):
                p_sb = st[g]["p_sb"]
                pT = wp.tile([128, JT, 128], BF16, tag="pT")
                for half in range(2):
                    tp = psB.tile([128, 512], BF16, tag="psB")
                    for k in range(4):
                        jc = half * 4 + k
                        nc.tensor.transpose(tp[:, ts(k, 128)],
                                            p_sb[:, ts(jc, 128)], ident_s[:])
                    nc.vector.tensor_copy(
                        pT[:].rearrange("p j c -> p (j c)")
                        [:, half * 512:(half + 1) * 512],
                        tp[:])
                pTc_ps = psB.tile([1, 128], F32, tag="psB")
                nc.tensor.matmul(pTc_ps[:], p_sb[:, HW:HW + 1], ident_s[:],
                                 start=True, stop=True)
                pTc = wp.tile([1, 128], BF16, tag="pTc")
                nc.vector.tensor_copy(pTc[:], pTc_ps[:])
                st[g]["pT"] = pT
                st[g]["pTc"] = pTc

            def wsum(g):
                xtb = xtb_all[g]
                pT = st[g]["pT"]
                pTc = st[g]["pTc"]
                w_ps = psD.tile([128, C], F32, tag="psD")
                for s in range(GRP):
                    for jc in range(JT):
                        nc.tensor.matmul(
                            w_ps[32 * s:32 * s + 32, :],
                            pT[:, jc, 32 * s:32 * s + 32], xtb[s][:, :, jc, :],
                            start=(jc == 0), stop=(jc == JT - 1),
                            tile_position=(0, 32 * s))
                nc.tensor.matmul(w_ps[:], pTc[:], pos0row_s[:],
                                 start=False, stop=True, skip_group_check=True)
                w_sb = wp.tile([128, C], BF16, tag="wsb")
                nc.vector.tensor_scalar_mul(w_sb[:], w_ps[:], st[g]["rz"][:])
                st[g]["w_sb"] = w_sb

            def wout(g):
                w_sb = st[g]["w_sb"]
                tp3 = psB.tile([128, 512], BF16, tag="psB")
                for mc in range(CT):
                    nc.tensor.transpose(tp3[:, ts(mc, 128)],
                                        w_sb[:, ts(mc, 128)], ident_s[:])
                wt_sb = wp.tile([128, CT, GRP, 32], BF16, tag="wtsb")
                nc.vector.tensor_copy(
                    wt_sb[:].rearrange("p t s u -> p (t s u)"), tp3[:])

                out_ps = psD.tile([128, CT, GRP, NH], F32, tag="psD")
                for mc in range(CT):
                    for tk in range(CT):
                        nc.tensor.matmul(
                            out_ps[:, mc], wvt2_s[:, tk, ts(mc, 128)],
                            wt_sb[:, tk, :, 0:NH],
                            start=(tk == 0), stop=(tk == CT - 1))
                om = wp.tile([128, CT, GRP, NH], F32, tag="om")
                nc.vector.tensor_mul(
                    om[:], out_ps[:],
                    mask_s[:, :, None, 0:NH].broadcast_to([128, CT, GRP, NH]))
                osel = wp.tile([128, CT, GRP], F32, tag="osel")
                nc.vector.tensor_reduce(osel[:], om[:], axis=AX.X, op=ALU.add)
                out_sb = wp.tile([128, CT, GRP], F32, tag="outsb")
                nc.vector.tensor_add(
                    out_sb[:], osel[:],
                    bv_s[:, :, None].broadcast_to([128, CT, GRP]))
                for s in range(GRP):
                    nc.sync.dma_start(
                        out=out_d[g * GRP + s].rearrange("(t p) -> p t", p=128),
                        in_=out_sb[:, :, s])

            # ---- software-pipelined emission of the two groups ----
            means(0)
            qm(0)
            means(1)
            scores(0)
            softmax(0)
            qm(1)
            scores(1)
            ptrans(0)
            wsum(0)
            softmax(1)
            wout(0)
            ptrans(1)
            wsum(1)
            wout(1)

    nc.compile()
    return nc


def _prep(pos_emb, Wq, bq, Wk, bk, Wv, bv):
    import ml_dtypes
    bf = ml_dtypes.bfloat16

    def ptn(v):  # [512] -> [128, CT], c = t*128 + p
        return np.ascontiguousarray(v.reshape(CT, 128).T)

    def chunkk(w):  # [512, N] -> [128, CT, N], k = t*128 + p
        return np.ascontiguousarray(w.reshape(CT, 128, -1).transpose(1, 0, 2))

    p1 = pos_emb[1:].sum(axis=0)
    pos0adj = (pos_emb[0] - p1 / HW).astype(np.float32)
    kpcv = (Wk.astype(np.float64) @ pos0adj.astype(np.float64)).astype(np.float32)
    mask = np.zeros((128, CT, 32), np.float32)
    for p in range(128):
        for t in range(CT):
            h = (t * 128 + p) // DH
            mask[p, t, h] = 1.0

    return {
        "wqt": chunkk(np.ascontiguousarray(Wq.T)).astype(bf),
        "wk": chunkk(Wk).astype(bf),
        "wvt2": chunkk(np.ascontiguousarray(Wv.T)).astype(bf),
        "kpc": ptn(kpcv).astype(bf),
        "pos0row": np.ascontiguousarray(pos0adj.reshape(1, C)).astype(bf),
        "pos0": ptn(pos0adj),
        "bqs": ptn(bq * ISQ * ALPHA),
        "bv": ptn(bv),
        "mask32": mask,
        "ident": np.eye(128, dtype=np.float32).astype(bf),
    }


def _make_in_maps(x, pos_emb, Wq, bq, Wk, bk, Wv, bv):
    import ml_dtypes
    bf = ml_dtypes.bfloat16
    f8 = ml_dtypes.float8_e4m3fn
    x = np.asarray(x, dtype=np.float32).reshape(B, C, HW)
    pos_emb = np.asarray(pos_emb, np.float32)
    shared = _prep(pos_emb, np.asarray(Wq, np.float32),
                   np.asarray(bq, np.float32), np.asarray(Wk, np.float32),
                   np.asarray(bk, np.float32), np.asarray(Wv, np.float32),
                   np.asarray(bv, np.float32))
    # fold pos_emb into the tokens: xt[c, j] = x[c, j] + pos_emb[1 + j, c]
    xp = x + pos_emb[1:].T[None, :, :]
    x8 = xp.astype(f8)
    x16 = xp.astype(bf)
    # xtr[b][p, t, jc, c'] = xp[b, t*128 + c', jc*128 + p]
    xtr = np.ascontiguousarray(
        x16.reshape(B, CT, 128, JT, 128).transpose(0, 4, 1, 3, 2))
    in_maps = []
    for i in range(NCORES):
        m = dict(shared)
        m["xs"] = np.ascontiguousarray(x8[i * BPC:(i + 1) * BPC])
        m["xtr"] = np.ascontiguousarray(xtr[i * BPC:(i + 1) * BPC])
        in_maps.append(m)
    return in_maps


def kernel(x, pos_emb, Wq, bq, Wk, bk, Wv, bv, num_heads):
    assert int(num_heads) == NH
    if "nc" not in _CACHE:
        _CACHE["nc"] = _build_nc()
    nc = _CACHE["nc"]
    in_maps = _make_in_maps(x, pos_emb, Wq, bq, Wk, bk, Wv, bv)
    res = run_bass_kernel_spmd(nc, in_maps, list(range(NCORES)))
    out = np.concatenate([res.results[i]["out"] for i in range(NCORES)], axis=0)
    return out.astype(np.float32)
